# revision 3
# baseline (speedup 1.0000x reference)
"""GNN message-passing kernel for Trainium2, SPMD over 8 NeuronCores.

Reference computation (N=8192 nodes, HID=128, OUT=64, E=262144 edges):
    h1  = x @ fc_w + fc_b
    h2a = relu(segsum(x) @ w1 + b1)         segsum = sum over in-edges (by dst)
    h2  = segsum(h2a) @ w2 + b2
    h   = (1-eps)*h1 + eps*h2
    ret = 0.5*(h @ h.T + x @ x.T)           [N, N]
    h_bn = batchnorm(h) * gamma + beta      [N, OUT]

Strategy:
  - segsum(v) @ w == segsum(v @ w): fold the linear maps in first, then do the
    neighbor sum as a dense matmul A @ y, where A[d, s] = multiplicity of edge
    s->d (host-built from src/dst, bf16-exact small ints).
  - Shard nodes in 8 contiguous blocks of 1024 (= rows of A, rows of outputs).
    Each core holds A_c^T (8192 x 1024, bf16) as 64 K-tiles and streams it on
    the PE twice (layer 1 and layer 2).
  - All on-chip activations live feature-major ([feat, node]) so matmul
    operands come out layout-compatible with no transposes anywhere.
  - Two AllGathers (bf16, 128/256 KB per rank) share h2a@w2 and h across
    cores; BN stats are computed redundantly per core from the gathered h.
  - ret tiles: PSUM-accumulate x-part (K=128) + h-part (K=64) in bf16,
    scale by 0.5 on the PSUM->SBUF copy, stream out row-block by row-block.
"""

import numpy as np
import ml_dtypes

import concourse.bass as bass
import concourse.bacc as bacc
import concourse.tile as tile
import concourse.mybir as mybir
from concourse import bass_utils

BF16 = mybir.dt.bfloat16
F32 = mybir.dt.float32
AF = mybir.ActivationFunctionType
ALU = mybir.AluOpType

N = 8192
HID = 128
OUT = 64
NCORES = 8
NL = N // NCORES          # 1024 nodes per core
KT = N // 128             # 64 K-tiles over the src axis
BN_EPS = 1e-5

_nb = ml_dtypes.bfloat16

_prog_cache = {}


def _build_program():
    """Build + compile the (identical-on-every-core) Bass program."""
    nc = bacc.Bacc(
        "TRN2",
        target_bir_lowering=False,
        debug=False,
        num_devices=NCORES,
    )

    # ---- kernel I/O ----
    xT_d = nc.dram_tensor("xT", [128, N], BF16, kind="ExternalInput")
    xTl_d = nc.dram_tensor("xTl", [128, NL], BF16, kind="ExternalInput")
    AT_d = nc.dram_tensor("AT", [128, KT, NL], BF16, kind="ExternalInput")
    w1_d = nc.dram_tensor("w1", [128, 128], BF16, kind="ExternalInput")
    fcw_d = nc.dram_tensor("fcw", [128, OUT], BF16, kind="ExternalInput")
    w2_d = nc.dram_tensor("w2", [128, OUT], BF16, kind="ExternalInput")
    b1_d = nc.dram_tensor("b1", [128, 1], F32, kind="ExternalInput")
    fcb_d = nc.dram_tensor("fcb", [OUT, 1], F32, kind="ExternalInput")
    b2_d = nc.dram_tensor("b2", [OUT, 1], F32, kind="ExternalInput")
    gamma_d = nc.dram_tensor("gamma", [OUT, 1], F32, kind="ExternalInput")
    beta_d = nc.dram_tensor("beta", [OUT, 1], F32, kind="ExternalInput")
    eps_d = nc.dram_tensor("eps", [OUT, NL], F32, kind="ExternalInput")

    ret_d = nc.dram_tensor("ret", [NL, N], F32, kind="ExternalOutput")
    hbnT_d = nc.dram_tensor("hbnT", [OUT, NL], F32, kind="ExternalOutput")

    with tile.TileContext(nc) as tc:
        with (
            tc.tile_pool(name="persist", bufs=1) as persist,
            tc.tile_pool(name="dram", bufs=1, space="DRAM") as dram,
        ):
            # ---- persistent SBUF loads ----
            A_sb = persist.tile([128, KT, NL], BF16)
            for g in range(8):
                nc.sync.dma_start(A_sb[:, g * 8:(g + 1) * 8, :],
                                  AT_d[:, g * 8:(g + 1) * 8, :])
            xT_sb = persist.tile([128, N], BF16)
            for g in range(4):
                nc.sync.dma_start(xT_sb[:, g * 2048:(g + 1) * 2048],
                                  xT_d[:, g * 2048:(g + 1) * 2048])
            xTl_sb = persist.tile([128, NL], BF16)
            nc.sync.dma_start(xTl_sb[:], xTl_d[:])
            w1_sb = persist.tile([128, 128], BF16)
            nc.sync.dma_start(w1_sb[:], w1_d[:])
            fcw_sb = persist.tile([128, OUT], BF16)
            nc.sync.dma_start(fcw_sb[:], fcw_d[:])
            w2_sb = persist.tile([128, OUT], BF16)
            nc.sync.dma_start(w2_sb[:], w2_d[:])
            b1_sb = persist.tile([128, 1], F32)
            nc.sync.dma_start(b1_sb[:], b1_d[:])
            fcb_sb = persist.tile([OUT, 1], F32)
            nc.sync.dma_start(fcb_sb[:], fcb_d[:])
            b2_sb = persist.tile([OUT, 1], F32)
            nc.sync.dma_start(b2_sb[:], b2_d[:])
            gamma_sb = persist.tile([OUT, 1], F32)
            nc.sync.dma_start(gamma_sb[:], gamma_d[:])
            beta_sb = persist.tile([OUT, 1], F32)
            nc.sync.dma_start(beta_sb[:], beta_d[:])
            eps_sb = persist.tile([OUT, NL], F32)
            nc.sync.dma_start(eps_sb[:], eps_d[:])

            hT_f32 = persist.tile([OUT, NL], F32)
            hT_bf = persist.tile([OUT, NL], BF16)
            hTf_sb = persist.tile([OUT, N], BF16)   # all-gathered h, feature-major

            # ---- phase 1: xw1[s, f2] = (x @ w1), node-major bf16 lhsT tiles ----
            with (
                tc.tile_pool(name="xw1pool", bufs=1) as xw1pool,
                tc.tile_pool(name="mid", bufs=1) as mid,
            ):
                xw1_sb = xw1pool.tile([128, KT, 128], BF16)
                with tc.tile_pool(name="ps_xw1", bufs=2, space="PSUM") as ps_xw1:
                    for m in range(KT):
                        p = ps_xw1.tile([128, 128], F32, tag="xw1p")
                        nc.tensor.matmul(p[:], xT_sb[:, m * 128:(m + 1) * 128],
                                         w1_sb[:], start=True, stop=True)
                        nc.any.tensor_copy(xw1_sb[:, m, :], p[:])

                # ---- phase 2 (L1): ns1T[f2, d] = (A @ x@w1)^T via K-tiles ----
                h2aT_sb = mid.tile([128, NL], BF16)
                with tc.tile_pool(name="ps_l1", bufs=1, space="PSUM") as ps_l1:
                    pns1 = [ps_l1.tile([128, 512], F32, tag=f"ns1_{n}",
                                       name=f"pns1_{n}") for n in range(2)]
                    for k in range(KT):
                        for n in range(2):
                            nc.tensor.matmul(
                                pns1[n][:], xw1_sb[:, k, :],
                                A_sb[:, k, n * 512:(n + 1) * 512],
                                start=(k == 0), stop=(k == KT - 1))
                    for n in range(2):
                        # h2a = relu(ns1 + b1), cast to bf16
                        nc.scalar.activation(h2aT_sb[:, n * 512:(n + 1) * 512],
                                             pns1[n][:], AF.Relu, bias=b1_sb[:])

                # ---- phase 3: h2aw2 (node-major, local rows) + AllGather ----
                h2aw2l_sb = mid.tile([128, NL // 128, OUT], BF16)
                with tc.tile_pool(name="ps_w2", bufs=2, space="PSUM") as ps_w2:
                    for m in range(NL // 128):
                        pw = ps_w2.tile([128, OUT], F32, tag="h2w2")
                        nc.tensor.matmul(pw[:],
                                         h2aT_sb[:, m * 128:(m + 1) * 128],
                                         w2_sb[:], start=True, stop=True)
                        nc.any.tensor_copy(h2aw2l_sb[:, m, :], pw[:])

                ag1_in = dram.tile([NL, OUT], BF16)
                ag1_out = dram.tile([N, OUT], BF16, addr_space="Shared")
                for m in range(NL // 128):
                    nc.sync.dma_start(ag1_in[m * 128:(m + 1) * 128, :],
                                      h2aw2l_sb[:, m, :])
                nc.gpsimd.collective_compute(
                    "AllGather", ALU.bypass,
                    replica_groups=[list(range(NCORES))],
                    ins=[ag1_in[:].opt()], outs=[ag1_out[:].opt()])

                h2aw2f_sb = mid.tile([128, KT, OUT], BF16)
                nc.sync.dma_start(
                    h2aw2f_sb[:],
                    ag1_out.rearrange("(k p) f -> p k f", p=128))

                # ---- phase 4 (L2 + h1 + mix) ----
                h1tmp = mid.tile([OUT, NL], F32)
                with tc.tile_pool(name="ps_l2", bufs=1, space="PSUM") as ps_l2:
                    pns2 = [ps_l2.tile([OUT, 512], F32, tag=f"ns2_{n}", name=f"pns2_{n}")
                            for n in range(2)]
                    for k in range(KT):
                        for n in range(2):
                            nc.tensor.matmul(
                                pns2[n][:], h2aw2f_sb[:, k, :],
                                A_sb[:, k, n * 512:(n + 1) * 512],
                                start=(k == 0), stop=(k == KT - 1))
                    ph1 = [ps_l2.tile([OUT, 512], F32, tag=f"h1_{n}", name=f"ph1_{n}")
                           for n in range(2)]
                    for n in range(2):
                        nc.tensor.matmul(ph1[n][:], fcw_sb[:],
                                         xTl_sb[:, n * 512:(n + 1) * 512],
                                         start=True, stop=True)

                    for n in range(2):
                        sl = slice(n * 512, (n + 1) * 512)
                        # h2 = ns2 + b2 ; h1 = x@fcw + fcb
                        nc.scalar.activation(hT_f32[:, sl], pns2[n][:],
                                             AF.Identity, bias=b2_sb[:])
                        nc.scalar.activation(h1tmp[:, sl], ph1[n][:],
                                             AF.Identity, bias=fcb_sb[:])
                        # h = h1 + eps*(h2 - h1)
                        nc.vector.tensor_sub(hT_f32[:, sl], hT_f32[:, sl],
                                             h1tmp[:, sl])
                        nc.vector.tensor_mul(hT_f32[:, sl], hT_f32[:, sl],
                                             eps_sb[:, sl])
                        nc.vector.tensor_add(hT_f32[:, sl], hT_f32[:, sl],
                                             h1tmp[:, sl])
                        nc.vector.tensor_copy(hT_bf[:, sl], hT_f32[:, sl])

            # ---- phase 5: AllGather h (feature-major rank stack) ----
            ag2_in = dram.tile([OUT, NL], BF16)
            ag2_out = dram.tile([NCORES * OUT, NL], BF16, addr_space="Shared")
            nc.sync.dma_start(ag2_in[:], hT_bf[:])
            nc.gpsimd.collective_compute(
                "AllGather", ALU.bypass,
                replica_groups=[list(range(NCORES))],
                ins=[ag2_in[:].opt()], outs=[ag2_out[:].opt()])
            nc.sync.dma_start(
                hTf_sb.rearrange("f (r n) -> f r n", r=NCORES),
                ag2_out.rearrange("(r f) n -> f r n", f=OUT))

            # ---- phase 6: BN stats (redundant on every core) + h_bn ----
            with tc.tile_pool(name="bn", bufs=1) as bn:
                ssq_parts = bn.tile([OUT, 8], F32)
                sq_pool_sb = bn.tile([OUT, 1024, 2], BF16, tag="sqscr")
                for i in range(8):
                    nc.scalar.activation(sq_pool_sb[:, :, i % 2],
                                         hTf_sb[:, i * 1024:(i + 1) * 1024],
                                         AF.Square,
                                         accum_out=ssq_parts[:, i:i + 1])
                sum_all = bn.tile([OUT, 1], F32)
                nc.vector.reduce_sum(sum_all[:], hTf_sb[:],
                                     axis=mybir.AxisListType.X)
                ssq = bn.tile([OUT, 1], F32)
                nc.vector.reduce_sum(ssq[:], ssq_parts[:],
                                     axis=mybir.AxisListType.X)

                mean = bn.tile([OUT, 1], F32)
                nc.vector.tensor_scalar_mul(mean[:], sum_all[:], 1.0 / N)
                var = bn.tile([OUT, 1], F32)
                nc.vector.tensor_scalar_mul(var[:], ssq[:], 1.0 / N)
                m2 = bn.tile([OUT, 1], F32)
                nc.vector.tensor_mul(m2[:], mean[:], mean[:])
                nc.vector.tensor_sub(var[:], var[:], m2[:])
                nc.vector.tensor_scalar_add(var[:], var[:], float(BN_EPS))
                sd = bn.tile([OUT, 1], F32)
                nc.scalar.sqrt(sd[:], var[:])
                inv = bn.tile([OUT, 1], F32)
                nc.vector.reciprocal(inv[:], sd[:])
                scale_bn = bn.tile([OUT, 1], F32)
                nc.vector.tensor_mul(scale_bn[:], inv[:], gamma_sb[:])
                bias_bn = bn.tile([OUT, 1], F32)
                nc.vector.tensor_mul(bias_bn[:], mean[:], scale_bn[:])
                nc.vector.tensor_sub(bias_bn[:], beta_sb[:], bias_bn[:])

                hbnT_sb = bn.tile([OUT, NL], F32)
                nc.vector.tensor_scalar(hbnT_sb[:], hT_f32[:],
                                        scalar1=scale_bn[:], scalar2=bias_bn[:],
                                        op0=ALU.mult, op1=ALU.add)
                nc.sync.dma_start(hbnT_d[:], hbnT_sb[:])

            # ---- phase 7: ret = 0.5*(h@h.T + x@x.T), row-block by row-block ----
            with (
                tc.tile_pool(name="stage", bufs=3) as stage_pool,
                tc.tile_pool(name="ps_ret", bufs=8, space="PSUM") as ps_ret,
            ):
                for m in range(NL // 128):
                    lhx = xTl_sb[:, m * 128:(m + 1) * 128]
                    lhh = hT_bf[:, m * 128:(m + 1) * 128]
                    for g in range(8):
                        stg = stage_pool.tile([128, 1024], F32, tag="stg")
                        for j in range(2):
                            ncol = g * 1024 + j * 512
                            pr = ps_ret.tile([128, 512], F32, tag="pr")
                            nc.tensor.matmul(pr[:], lhx,
                                             xT_sb[:, ncol:ncol + 512],
                                             start=True, stop=False)
                            nc.tensor.matmul(pr[:], lhh,
                                             hTf_sb[:, ncol:ncol + 512],
                                             start=False, stop=True)
                            nc.any.tensor_scalar_mul(
                                stg[:, j * 512:(j + 1) * 512], pr[:], 0.5)
                        nc.sync.dma_start(
                            ret_d[m * 128:(m + 1) * 128,
                                  g * 1024:(g + 1) * 1024], stg[:])

    nc.compile()
    return nc


def _get_program():
    if "nc" not in _prog_cache:
        _prog_cache["nc"] = _build_program()
    return _prog_cache["nc"]


def _host_prep(x, src, dst, fc_w, fc_b, w1, w1_b, w2, w2_b, epsilon,
               gamma, beta):
    """Build the per-core input maps (all device-side preprocessing)."""
    x = np.asarray(x, np.float32)
    src = np.asarray(src).astype(np.int64)
    dst = np.asarray(dst).astype(np.int64)

    counts = np.bincount(dst * N + src, minlength=N * N)
    A = counts.reshape(N, N).astype(_nb)   # A[d, s] = edge multiplicity s->d
    del counts

    xT = np.ascontiguousarray(x.T.astype(_nb))          # [128, N]
    w1b = np.ascontiguousarray(np.asarray(w1, np.float32).astype(_nb))
    fcwb = np.ascontiguousarray(np.asarray(fc_w, np.float32).astype(_nb))
    w2b = np.ascontiguousarray(np.asarray(w2, np.float32).astype(_nb))
    b1 = np.ascontiguousarray(np.asarray(w1_b, np.float32).reshape(128, 1))
    fcb = np.ascontiguousarray(np.asarray(fc_b, np.float32).reshape(OUT, 1))
    b2 = np.ascontiguousarray(np.asarray(w2_b, np.float32).reshape(OUT, 1))
    gam = np.ascontiguousarray(np.asarray(gamma, np.float32).reshape(OUT, 1))
    bet = np.ascontiguousarray(np.asarray(beta, np.float32).reshape(OUT, 1))
    epsilon = np.asarray(epsilon, np.float32)

    in_maps = []
    for c in range(NCORES):
        rows = slice(c * NL, (c + 1) * NL)
        # A_c^T laid out [p, k, d] = A[c*NL + d, k*128 + p]
        ATc = np.ascontiguousarray(
            A[rows, :].T.reshape(KT, 128, NL).transpose(1, 0, 2))
        eps_c = np.ascontiguousarray(
            np.broadcast_to(epsilon[rows][None, :], (OUT, NL)).astype(np.float32))
        in_maps.append({
            "xT": xT,
            "xTl": np.ascontiguousarray(xT[:, rows]),
            "AT": ATc,
            "w1": w1b, "fcw": fcwb, "w2": w2b,
            "b1": b1, "fcb": fcb, "b2": b2,
            "gamma": gam, "beta": bet,
            "eps": eps_c,
        })
    return in_maps


def _assemble(results):
    ret = np.concatenate(
        [np.asarray(results[c]["ret"], np.float32) for c in range(NCORES)],
        axis=0)
    h_bn = np.concatenate(
        [np.asarray(results[c]["hbnT"], np.float32) for c in range(NCORES)],
        axis=1).T.copy()
    return ret, h_bn


def kernel(x, adj, src, dst, fc_w, fc_b, w1, w1_b, w2, w2_b, epsilon,
           gamma, beta):
    nc = _get_program()
    in_maps = _host_prep(x, src, dst, fc_w, fc_b, w1, w1_b, w2, w2_b,
                         epsilon, gamma, beta)
    res = bass_utils.run_bass_kernel_spmd(nc, in_maps,
                                          core_ids=list(range(NCORES)))
    return _assemble(res.results)


# revision 4
# speedup vs baseline: 1.3776x; 1.3776x over previous
"""GNN message-passing kernel for Trainium2, SPMD over 8 NeuronCores.

Reference computation (N=8192 nodes, HID=128, OUT=64, E=262144 edges):
    h1  = x @ fc_w + fc_b
    h2a = relu(segsum(x) @ w1 + b1)         segsum = sum over in-edges (by dst)
    h2  = segsum(h2a) @ w2 + b2
    h   = (1-eps)*h1 + eps*h2
    ret = 0.5*(h @ h.T + x @ x.T)           [N, N]
    h_bn = batchnorm(h) * gamma + beta      [N, OUT]

Strategy:
  - segsum(v) @ w == segsum(v @ w): fold the linear maps in first, then do the
    neighbor sum as a dense matmul A @ y, where A[d, s] = multiplicity of edge
    s->d (host-built from src/dst, bf16-exact small ints).
  - Shard nodes in 8 contiguous blocks of 1024 (= rows of A, rows of outputs).
    Each core holds A_c^T (8192 x 1024, bf16) as 64 K-tiles and streams it on
    the PE twice (layer 1 and layer 2).
  - All on-chip activations live feature-major ([feat, node]) so matmul
    operands come out layout-compatible with no transposes anywhere.
  - Two AllGathers (bf16, 128/256 KB per rank) share h2a@w2 and h across
    cores; BN stats are computed redundantly per core from the gathered h.
  - ret tiles: PSUM-accumulate x-part (K=128) + h-part (K=64) in bf16,
    scale by 0.5 on the PSUM->SBUF copy, stream out row-block by row-block.
"""

import numpy as np
import ml_dtypes

import concourse.bass as bass
import concourse.bacc as bacc
import concourse.tile as tile
import concourse.mybir as mybir
from concourse import bass_utils

BF16 = mybir.dt.bfloat16
F32 = mybir.dt.float32
AF = mybir.ActivationFunctionType
ALU = mybir.AluOpType

N = 8192
HID = 128
OUT = 64
NCORES = 8
NL = N // NCORES          # 1024 nodes per core
KT = N // 128             # 64 K-tiles over the src axis
BN_EPS = 1e-5

_nb = ml_dtypes.bfloat16

_prog_cache = {}


def _build_program():
    """Build + compile the (identical-on-every-core) Bass program."""
    nc = bacc.Bacc(
        "TRN2",
        target_bir_lowering=False,
        debug=False,
        num_devices=NCORES,
    )

    # ---- kernel I/O ----
    xT_d = nc.dram_tensor("xT", [128, N], BF16, kind="ExternalInput")
    xTl_d = nc.dram_tensor("xTl", [128, NL], BF16, kind="ExternalInput")
    AT_d = nc.dram_tensor("AT", [128, KT, NL], BF16, kind="ExternalInput")
    w1_d = nc.dram_tensor("w1", [128, 128], BF16, kind="ExternalInput")
    fcw_d = nc.dram_tensor("fcw", [128, OUT], BF16, kind="ExternalInput")
    w2_d = nc.dram_tensor("w2", [128, OUT], BF16, kind="ExternalInput")
    b1_d = nc.dram_tensor("b1", [128, 1], F32, kind="ExternalInput")
    fcb_d = nc.dram_tensor("fcb", [OUT, 1], F32, kind="ExternalInput")
    b2_d = nc.dram_tensor("b2", [OUT, 1], F32, kind="ExternalInput")
    gamma_d = nc.dram_tensor("gamma", [OUT, 1], F32, kind="ExternalInput")
    beta_d = nc.dram_tensor("beta", [OUT, 1], F32, kind="ExternalInput")
    eps_d = nc.dram_tensor("eps", [OUT, NL], F32, kind="ExternalInput")

    ret_d = nc.dram_tensor("ret", [NL, N], BF16, kind="ExternalOutput")
    hbnT_d = nc.dram_tensor("hbnT", [OUT, NL], F32, kind="ExternalOutput")

    with tile.TileContext(nc) as tc:
        with (
            tc.tile_pool(name="persist", bufs=1) as persist,
            tc.tile_pool(name="dram", bufs=1, space="DRAM") as dram,
        ):
            # ---- persistent SBUF loads ----
            # Small/early tensors go on the scalar HWDGE ring so they are not
            # queued behind the big A load (sync ring); A streams in 16 chunks
            # so L1 can consume K-tiles as they land.
            xT_sb = persist.tile([128, N], BF16)
            for g in range(4):
                nc.scalar.dma_start(xT_sb[:, g * 2048:(g + 1) * 2048],
                                    xT_d[:, g * 2048:(g + 1) * 2048])
            w1_sb = persist.tile([128, 128], BF16)
            nc.scalar.dma_start(w1_sb[:], w1_d[:])
            xTl_sb = persist.tile([128, NL], BF16)
            nc.scalar.dma_start(xTl_sb[:], xTl_d[:])
            fcw_sb = persist.tile([128, OUT], BF16)
            nc.scalar.dma_start(fcw_sb[:], fcw_d[:])
            w2_sb = persist.tile([128, OUT], BF16)
            nc.scalar.dma_start(w2_sb[:], w2_d[:])
            b1_sb = persist.tile([128, 1], F32)
            nc.scalar.dma_start(b1_sb[:], b1_d[:])
            fcb_sb = persist.tile([OUT, 1], F32)
            nc.scalar.dma_start(fcb_sb[:], fcb_d[:])
            b2_sb = persist.tile([OUT, 1], F32)
            nc.scalar.dma_start(b2_sb[:], b2_d[:])
            gamma_sb = persist.tile([OUT, 1], F32)
            nc.scalar.dma_start(gamma_sb[:], gamma_d[:])
            beta_sb = persist.tile([OUT, 1], F32)
            nc.scalar.dma_start(beta_sb[:], beta_d[:])
            eps_sb = persist.tile([OUT, NL], F32)
            nc.scalar.dma_start(eps_sb[:], eps_d[:])
            A_sb = persist.tile([128, KT, NL], BF16)
            for g in range(16):
                nc.sync.dma_start(A_sb[:, g * 4:(g + 1) * 4, :],
                                  AT_d[:, g * 4:(g + 1) * 4, :])

            hT_f32 = persist.tile([OUT, NL], F32)
            hT_bf = persist.tile([OUT, NL], BF16)
            hTf_sb = persist.tile([OUT, N], BF16)   # all-gathered h, feature-major

            # ---- phase 1: xw1[s, f2] = (x @ w1), node-major bf16 lhsT tiles ----
            with (
                tc.tile_pool(name="xw1pool", bufs=1) as xw1pool,
                tc.tile_pool(name="mid", bufs=1) as mid,
            ):
                xw1_sb = xw1pool.tile([128, KT, 128], BF16)
                with tc.tile_pool(name="ps_xw1", bufs=2, space="PSUM") as ps_xw1:
                    for m in range(KT):
                        p = ps_xw1.tile([128, 128], F32, tag="xw1p")
                        nc.tensor.matmul(p[:], xT_sb[:, m * 128:(m + 1) * 128],
                                         w1_sb[:], start=True, stop=True)
                        nc.any.tensor_copy(xw1_sb[:, m, :], p[:])

                # ---- phase 2 (L1): ns1T[f2, d] = (A @ x@w1)^T via K-tiles ----
                h2aT_sb = mid.tile([128, NL], BF16)
                with tc.tile_pool(name="ps_l1", bufs=1, space="PSUM") as ps_l1:
                    pns1 = [ps_l1.tile([128, 512], F32, tag=f"ns1_{n}",
                                       name=f"pns1_{n}") for n in range(2)]
                    for k in range(KT):
                        for n in range(2):
                            nc.tensor.matmul(
                                pns1[n][:], xw1_sb[:, k, :],
                                A_sb[:, k, n * 512:(n + 1) * 512],
                                start=(k == 0), stop=(k == KT - 1))
                    for n in range(2):
                        # h2a = relu(ns1 + b1), cast to bf16
                        nc.scalar.activation(h2aT_sb[:, n * 512:(n + 1) * 512],
                                             pns1[n][:], AF.Relu, bias=b1_sb[:])

                # ---- phase 3: h2aw2 (node-major, local rows) + AllGather ----
                h2aw2l_sb = mid.tile([128, NL // 128, OUT], BF16)
                with tc.tile_pool(name="ps_w2", bufs=2, space="PSUM") as ps_w2:
                    for m in range(NL // 128):
                        pw = ps_w2.tile([128, OUT], F32, tag="h2w2")
                        nc.tensor.matmul(pw[:],
                                         h2aT_sb[:, m * 128:(m + 1) * 128],
                                         w2_sb[:], start=True, stop=True)
                        nc.any.tensor_copy(h2aw2l_sb[:, m, :], pw[:])

                ag1_in = dram.tile([NL, OUT], BF16)
                ag1_out = dram.tile([N, OUT], BF16, addr_space="Shared")
                nc.sync.dma_start(
                    ag1_in.rearrange("(m p) f -> p m f", p=128),
                    h2aw2l_sb[:])
                nc.gpsimd.collective_compute(
                    "AllGather", ALU.bypass,
                    replica_groups=[list(range(NCORES))],
                    ins=[ag1_in[:].opt()], outs=[ag1_out[:].opt()])

                h2aw2f_sb = mid.tile([128, KT, OUT], BF16)
                nc.sync.dma_start(
                    h2aw2f_sb[:],
                    ag1_out.rearrange("(k p) f -> p k f", p=128))

                # ---- phase 4 (L2 + h1 + mix) ----
                h1tmp = mid.tile([OUT, NL], F32)
                with tc.tile_pool(name="ps_l2", bufs=1, space="PSUM") as ps_l2:
                    pns2 = [ps_l2.tile([OUT, 512], F32, tag=f"ns2_{n}", name=f"pns2_{n}")
                            for n in range(2)]
                    for k in range(KT):
                        for n in range(2):
                            nc.tensor.matmul(
                                pns2[n][:], h2aw2f_sb[:, k, :],
                                A_sb[:, k, n * 512:(n + 1) * 512],
                                start=(k == 0), stop=(k == KT - 1))
                    ph1 = [ps_l2.tile([OUT, 512], F32, tag=f"h1_{n}", name=f"ph1_{n}")
                           for n in range(2)]
                    for n in range(2):
                        nc.tensor.matmul(ph1[n][:], fcw_sb[:],
                                         xTl_sb[:, n * 512:(n + 1) * 512],
                                         start=True, stop=True)

                    for n in range(2):
                        sl = slice(n * 512, (n + 1) * 512)
                        # h2 = ns2 + b2 ; h1 = x@fcw + fcb
                        nc.scalar.activation(hT_f32[:, sl], pns2[n][:],
                                             AF.Identity, bias=b2_sb[:])
                        nc.scalar.activation(h1tmp[:, sl], ph1[n][:],
                                             AF.Identity, bias=fcb_sb[:])
                        # h = h1 + eps*(h2 - h1)
                        nc.vector.tensor_sub(hT_f32[:, sl], hT_f32[:, sl],
                                             h1tmp[:, sl])
                        nc.vector.tensor_mul(hT_f32[:, sl], hT_f32[:, sl],
                                             eps_sb[:, sl])
                        nc.vector.tensor_add(hT_f32[:, sl], hT_f32[:, sl],
                                             h1tmp[:, sl])
                        nc.vector.tensor_copy(hT_bf[:, sl], hT_f32[:, sl])

            # ---- phase 5: AllGather h (feature-major rank stack) ----
            ag2_in = dram.tile([OUT, NL], BF16)
            ag2_out = dram.tile([NCORES * OUT, NL], BF16, addr_space="Shared")
            nc.sync.dma_start(ag2_in[:], hT_bf[:])
            nc.gpsimd.collective_compute(
                "AllGather", ALU.bypass,
                replica_groups=[list(range(NCORES))],
                ins=[ag2_in[:].opt()], outs=[ag2_out[:].opt()])
            nc.sync.dma_start(
                hTf_sb.rearrange("f (r n) -> f r n", r=NCORES),
                ag2_out.rearrange("(r f) n -> f r n", f=OUT))

            # ---- phase 6: BN stats (redundant on every core) + h_bn ----
            with tc.tile_pool(name="bn", bufs=1) as bn:
                ssq_parts = bn.tile([OUT, 8], F32)
                sq_pool_sb = bn.tile([OUT, 1024, 2], BF16, tag="sqscr")
                for i in range(8):
                    nc.scalar.activation(sq_pool_sb[:, :, i % 2],
                                         hTf_sb[:, i * 1024:(i + 1) * 1024],
                                         AF.Square,
                                         accum_out=ssq_parts[:, i:i + 1])
                sum_all = bn.tile([OUT, 1], F32)
                nc.vector.reduce_sum(sum_all[:], hTf_sb[:],
                                     axis=mybir.AxisListType.X)
                ssq = bn.tile([OUT, 1], F32)
                nc.vector.reduce_sum(ssq[:], ssq_parts[:],
                                     axis=mybir.AxisListType.X)

                mean = bn.tile([OUT, 1], F32)
                nc.vector.tensor_scalar_mul(mean[:], sum_all[:], 1.0 / N)
                var = bn.tile([OUT, 1], F32)
                nc.vector.tensor_scalar_mul(var[:], ssq[:], 1.0 / N)
                m2 = bn.tile([OUT, 1], F32)
                nc.vector.tensor_mul(m2[:], mean[:], mean[:])
                nc.vector.tensor_sub(var[:], var[:], m2[:])
                nc.vector.tensor_scalar_add(var[:], var[:], float(BN_EPS))
                sd = bn.tile([OUT, 1], F32)
                nc.scalar.sqrt(sd[:], var[:])
                inv = bn.tile([OUT, 1], F32)
                nc.vector.reciprocal(inv[:], sd[:])
                scale_bn = bn.tile([OUT, 1], F32)
                nc.vector.tensor_mul(scale_bn[:], inv[:], gamma_sb[:])
                bias_bn = bn.tile([OUT, 1], F32)
                nc.vector.tensor_mul(bias_bn[:], mean[:], scale_bn[:])
                nc.vector.tensor_sub(bias_bn[:], beta_sb[:], bias_bn[:])

                hbnT_sb = bn.tile([OUT, NL], F32)
                nc.vector.tensor_scalar(hbnT_sb[:], hT_f32[:],
                                        scalar1=scale_bn[:], scalar2=bias_bn[:],
                                        op0=ALU.mult, op1=ALU.add)
                nc.sync.dma_start(hbnT_d[:], hbnT_sb[:])

            # ---- phase 7: ret = 0.5*(h@h.T + x@x.T), row-block by row-block ----
            # Per (m, half): 8 PSUM banks accumulate the x-part with one
            # weight load, then the h-part with one more; copies split
            # between DVE and ACT; 1 MB bf16 stores alternate HWDGE rings.
            with (
                tc.tile_pool(name="stage", bufs=4) as stage_pool,
                tc.tile_pool(name="ps_ret", bufs=8, space="PSUM") as ps_ret,
            ):
                for m in range(NL // 128):
                    lhx = xTl_sb[:, m * 128:(m + 1) * 128]
                    lhh = hT_bf[:, m * 128:(m + 1) * 128]
                    for half in range(2):
                        base = half * 4096
                        prs = [ps_ret.tile([128, 512], F32, tag="pr",
                                           name=f"pr_{m}_{half}_{j}")
                               for j in range(8)]
                        for j in range(8):
                            nc.tensor.matmul(prs[j][:], lhx,
                                             xT_sb[:, base + j * 512:
                                                   base + (j + 1) * 512],
                                             start=True, stop=False)
                        for j in range(8):
                            nc.tensor.matmul(prs[j][:], lhh,
                                             hTf_sb[:, base + j * 512:
                                                    base + (j + 1) * 512],
                                             start=False, stop=True)
                        for q in range(2):
                            stg = stage_pool.tile([128, 2048], BF16, tag="stg",
                                                  name=f"stg_{m}_{half}_{q}")
                            for j in range(4):
                                dst = stg[:, j * 512:(j + 1) * 512]
                                srcp = prs[q * 4 + j][:]
                                if j % 2 == 0:
                                    nc.vector.tensor_scalar_mul(dst, srcp, 0.5)
                                else:
                                    nc.scalar.mul(dst, srcp, 0.5)
                            eng = nc.sync if (half * 2 + q) % 2 == 0 else nc.scalar
                            eng.dma_start(
                                ret_d[m * 128:(m + 1) * 128,
                                      base + q * 2048:base + (q + 1) * 2048],
                                stg[:])

    nc.compile()
    return nc


def _get_program():
    if "nc" not in _prog_cache:
        _prog_cache["nc"] = _build_program()
    return _prog_cache["nc"]


def _host_prep(x, src, dst, fc_w, fc_b, w1, w1_b, w2, w2_b, epsilon,
               gamma, beta):
    """Build the per-core input maps (all device-side preprocessing)."""
    x = np.asarray(x, np.float32)
    src = np.asarray(src).astype(np.int64)
    dst = np.asarray(dst).astype(np.int64)

    counts = np.bincount(dst * N + src, minlength=N * N)
    A = counts.reshape(N, N).astype(_nb)   # A[d, s] = edge multiplicity s->d
    del counts

    xT = np.ascontiguousarray(x.T.astype(_nb))          # [128, N]
    w1b = np.ascontiguousarray(np.asarray(w1, np.float32).astype(_nb))
    fcwb = np.ascontiguousarray(np.asarray(fc_w, np.float32).astype(_nb))
    w2b = np.ascontiguousarray(np.asarray(w2, np.float32).astype(_nb))
    b1 = np.ascontiguousarray(np.asarray(w1_b, np.float32).reshape(128, 1))
    fcb = np.ascontiguousarray(np.asarray(fc_b, np.float32).reshape(OUT, 1))
    b2 = np.ascontiguousarray(np.asarray(w2_b, np.float32).reshape(OUT, 1))
    gam = np.ascontiguousarray(np.asarray(gamma, np.float32).reshape(OUT, 1))
    bet = np.ascontiguousarray(np.asarray(beta, np.float32).reshape(OUT, 1))
    epsilon = np.asarray(epsilon, np.float32)

    in_maps = []
    for c in range(NCORES):
        rows = slice(c * NL, (c + 1) * NL)
        # A_c^T laid out [p, k, d] = A[c*NL + d, k*128 + p]
        ATc = np.ascontiguousarray(
            A[rows, :].T.reshape(KT, 128, NL).transpose(1, 0, 2))
        eps_c = np.ascontiguousarray(
            np.broadcast_to(epsilon[rows][None, :], (OUT, NL)).astype(np.float32))
        in_maps.append({
            "xT": xT,
            "xTl": np.ascontiguousarray(xT[:, rows]),
            "AT": ATc,
            "w1": w1b, "fcw": fcwb, "w2": w2b,
            "b1": b1, "fcb": fcb, "b2": b2,
            "gamma": gam, "beta": bet,
            "eps": eps_c,
        })
    return in_maps


def _assemble(results):
    ret = np.concatenate(
        [np.asarray(results[c]["ret"]).astype(np.float32)
         for c in range(NCORES)],
        axis=0)
    h_bn = np.concatenate(
        [np.asarray(results[c]["hbnT"], np.float32) for c in range(NCORES)],
        axis=1).T.copy()
    return ret, h_bn


def kernel(x, adj, src, dst, fc_w, fc_b, w1, w1_b, w2, w2_b, epsilon,
           gamma, beta):
    nc = _get_program()
    in_maps = _host_prep(x, src, dst, fc_w, fc_b, w1, w1_b, w2, w2_b,
                         epsilon, gamma, beta)
    res = bass_utils.run_bass_kernel_spmd(nc, in_maps,
                                          core_ids=list(range(NCORES)))
    return _assemble(res.results)
